# revision 3
# baseline (speedup 1.0000x reference)
"""DeepseekV3 MoE layer on 8 Trainium2 NeuronCores (Bass/Tile).

Strategy (expert-parallel, host-routed):
  - Router (h @ gate_w.T, sigmoid, top-8-of-16, weight norm) runs on host in
    fp32 — it is ~134 MFLOP, negligible, and data-dependent control flow is
    hostile to the static device ISA.
  - The 16 routed experts are sharded 2-per-core.  The host gathers each
    expert's tokens (capacity-padded to a multiple of 128), ships them
    transposed ([H, C] "feature-major") in bf16, and the device runs the
    fused SwiGLU chain  Y = (silu(X@Wg) * (X@Wu)) @ Wd  with fp32 PSUM
    accumulation, scaling rows by the combine weight on the way out.
  - The shared expert is token-parallel: each core computes the full shared
    MLP for its T/8 = 256 tokens (shared weights replicated, streamed).
  - Host scatters per-expert outputs back (indices are unique per expert)
    and adds the shared part.  All combination arithmetic is fp32.

Device layout notes:
  - All matmuls keep activations feature-major [feat, tokens] so that the
    gate/up projections need no transposes (weights are naturally [K, M])
    and the down projection consumes A^T directly as the stationary operand,
    returning token-major output tiles.
  - PSUM is managed as 8 single-bank [128, 512] fp32 tiles shared by every
    accumulation group.
"""

import math

import numpy as np
import ml_dtypes

import concourse.mybir as mybir
import concourse.tile as tile
from concourse import bacc
from concourse.bass_utils import run_bass_kernel_spmd

BF16 = ml_dtypes.bfloat16
F32 = mybir.dt.float32
BF = mybir.dt.bfloat16
ACT = mybir.ActivationFunctionType

H = 2048          # hidden size
IM = 1024         # routed expert intermediate
E = 16            # routed experts
TOPK = 8
T = 2048          # tokens (B=1, L=2048)
SCALE = 2.5
NCORES = 8
EL = E // NCORES  # experts per core
IS = 2048         # shared expert intermediate (IM * n_shared)
TSH = T // NCORES # shared-expert tokens per core
P = 128


def _chunks(total, size):
    return [(o, min(size, total - o)) for o in range(0, total, size)]


def build_program(C, h=H, im=IM, ishr=IS, tsh=TSH, el=EL, n_devices=NCORES):
    """Build + bass-compile the per-core SPMD program for capacity C."""
    assert C % P == 0 and h % 512 == 0 and im % P == 0 and ishr % P == 0
    KH = h // P            # contraction tiles over hidden dim
    MI = im // P           # routed intermediate partition tiles (<= 8)
    assert MI <= 8
    CT = C // P
    HN = h // 512
    ISM = ishr // P        # shared intermediate partition tiles
    nch = _chunks(C, 512)
    assert tsh <= 512 and tsh % P == 0

    nc = bacc.Bacc("TRN2", target_bir_lowering=False, debug=False,
                   num_devices=n_devices)

    xt = nc.dram_tensor("xt", [h, el * C], BF, kind="ExternalInput").ap()
    wtk = nc.dram_tensor("wtk", [P, el * CT], F32, kind="ExternalInput").ap()
    hts = nc.dram_tensor("hts", [h, tsh], BF, kind="ExternalInput").ap()
    wg = nc.dram_tensor("wg", [el, h, im], BF, kind="ExternalInput").ap()
    wu = nc.dram_tensor("wu", [el, h, im], BF, kind="ExternalInput").ap()
    wd = nc.dram_tensor("wd", [el, im, h], BF, kind="ExternalInput").ap()
    swg = nc.dram_tensor("swg", [h, ishr], BF, kind="ExternalInput").ap()
    swu = nc.dram_tensor("swu", [h, ishr], BF, kind="ExternalInput").ap()
    swd = nc.dram_tensor("swd", [ishr, h], BF, kind="ExternalInput").ap()
    yex = nc.dram_tensor("yex", [el, C, h], F32, kind="ExternalOutput").ap()
    ysh = nc.dram_tensor("ysh", [tsh, h], F32, kind="ExternalOutput").ap()

    with tile.TileContext(nc) as tc:
        with (
            tc.tile_pool(name="consts", bufs=1) as consts,
            tc.tile_pool(name="wpool", bufs=2) as wpool,
            tc.tile_pool(name="xpool", bufs=6) as xpool,
            tc.tile_pool(name="gpool", bufs=1) as gpool,
            tc.tile_pool(name="apool", bufs=2) as apool,
            tc.tile_pool(name="shw", bufs=3) as shw,
            tc.tile_pool(name="stage", bufs=6) as stage,
            tc.tile_pool(name="psum", bufs=8, space="PSUM") as psum,
        ):
            wtk_sb = consts.tile([P, el * CT], F32, name="wtk_sb")
            nc.sync.dma_start(wtk_sb[:], wtk[:, :])

            # ---------------- shared expert (token-parallel) --------------
            hts_sb = consts.tile([P, KH, tsh], BF, name="hts_sb")
            nc.sync.dma_start(hts_sb[:], hts.rearrange("(k p) t -> p k t", p=P))

            gss = consts.tile([P, ISM, tsh], BF, name="gss")
            ass = consts.tile([P, ISM, tsh], BF, name="ass")

            mgroups = _chunks(ISM, 8)
            for mat, is_gate in ((swg, True), (swu, False)):
                for (moff, msz) in mgroups:
                    pss = [psum.tile([P, 512], F32, name="ps", tag="ps")
                           for _ in range(msz)]
                    for k in range(KH):
                        wsl = shw.tile([P, 8 * P], BF, name="wsl")[:, :msz * P]
                        nc.sync.dma_start(
                            wsl,
                            mat[k * P:(k + 1) * P,
                                moff * P:moff * P + msz * P])
                        for m in range(msz):
                            nc.tensor.matmul(
                                pss[m][:, :tsh],
                                wsl[:, m * P:(m + 1) * P],
                                hts_sb[:, k, :],
                                start=(k == 0), stop=(k == KH - 1))
                    for m in range(msz):
                        mi = moff + m
                        if is_gate:
                            nc.scalar.activation(gss[:, mi, :],
                                                 pss[m][:, :tsh], ACT.Silu)
                        else:
                            nc.vector.tensor_mul(ass[:, mi, :], gss[:, mi, :],
                                                 pss[m][:, :tsh])

            # down-projection of the shared expert: token-major out
            psy = [[psum.tile([P, 512], F32, name="ps", tag="ps")
                    for _ in range(HN)] for _ in range(tsh // P)]
            for kk in range(ISM):
                dsl = shw.tile([P, h], BF, name="dsl")
                nc.sync.dma_start(dsl[:], swd[kk * P:(kk + 1) * P, :])
                for cm in range(tsh // P):
                    for hn in range(HN):
                        nc.tensor.matmul(
                            psy[cm][hn], ass[:, kk, cm * P:(cm + 1) * P],
                            dsl[:, hn * 512:(hn + 1) * 512],
                            start=(kk == 0), stop=(kk == ISM - 1))
            for cm in range(tsh // P):
                for hn in range(HN):
                    st = stage.tile([P, 512], F32, name="st")
                    nc.scalar.activation(st[:], psy[cm][hn][:], ACT.Copy)
                    nc.sync.dma_start(
                        ysh[cm * P:(cm + 1) * P, hn * 512:(hn + 1) * 512],
                        st[:])

            # ---------------- routed experts (expert-parallel) ------------
            for e in range(el):
                # gate projection
                wgt = wpool.tile([P, KH, im], BF, name="wmat", tag="wmat")
                nc.sync.dma_start(wgt[:],
                                  wg[e].rearrange("(k p) i -> p k i", p=P))
                gst = gpool.tile([P, MI, C], BF, name="gst")
                for (noff, nsz) in nch:
                    pse = [psum.tile([P, 512], F32, name="ps", tag="ps")
                           for _ in range(MI)]
                    for k in range(KH):
                        xsl = xpool.tile([P, 512], BF, name="xsl")[:, :nsz]
                        nc.sync.dma_start(
                            xsl, xt[k * P:(k + 1) * P,
                                    e * C + noff:e * C + noff + nsz])
                        for m in range(MI):
                            nc.tensor.matmul(
                                pse[m][:, :nsz], wgt[:, k, m * P:(m + 1) * P],
                                xsl, start=(k == 0), stop=(k == KH - 1))
                    for m in range(MI):
                        nc.scalar.activation(gst[:, m, noff:noff + nsz],
                                             pse[m][:, :nsz], ACT.Silu)

                # up projection (evicts through silu(G) * U)
                wut = wpool.tile([P, KH, im], BF, name="wmat", tag="wmat")
                nc.sync.dma_start(wut[:],
                                  wu[e].rearrange("(k p) i -> p k i", p=P))
                ast = apool.tile([P, MI, C], BF, name="ast")
                for (noff, nsz) in nch:
                    pse = [psum.tile([P, 512], F32, name="ps", tag="ps")
                           for _ in range(MI)]
                    for k in range(KH):
                        xsl = xpool.tile([P, 512], BF, name="xsl")[:, :nsz]
                        nc.sync.dma_start(
                            xsl, xt[k * P:(k + 1) * P,
                                    e * C + noff:e * C + noff + nsz])
                        for m in range(MI):
                            nc.tensor.matmul(
                                pse[m][:, :nsz], wut[:, k, m * P:(m + 1) * P],
                                xsl, start=(k == 0), stop=(k == KH - 1))
                    for m in range(MI):
                        nc.vector.tensor_mul(ast[:, m, noff:noff + nsz],
                                             gst[:, m, noff:noff + nsz],
                                             pse[m][:, :nsz])

                # down projection, token-major out, combine-weight scaling
                wdt = wpool.tile([P, MI, h], BF, name="wmat", tag="wmat")
                nc.sync.dma_start(wdt[:],
                                  wd[e].rearrange("(k p) i -> p k i", p=P))
                for cm in range(CT):
                    for hn in range(HN):
                        pst = psum.tile([P, 512], F32, name="ps", tag="ps")
                        for kk in range(MI):
                            nc.tensor.matmul(
                                pst, ast[:, kk, cm * P:(cm + 1) * P],
                                wdt[:, kk, hn * 512:(hn + 1) * 512],
                                start=(kk == 0), stop=(kk == MI - 1))
                        st = stage.tile([P, 512], F32, name="st")
                        nc.scalar.activation(
                            st[:], pst[:], ACT.Copy,
                            scale=wtk_sb[:, e * CT + cm:e * CT + cm + 1])
                        nc.sync.dma_start(
                            yex[e, cm * P:(cm + 1) * P,
                                hn * 512:(hn + 1) * 512], st[:])

    nc.compile()
    return nc


_prog_cache = {}

# Debug/timing hooks for the local test harness (harmless defaults for
# grading: no tracing, results kept only when asked for).
TRACE = False
TRACE_KWARGS = {}
LAST_RESULT = None


def _get_program(C):
    if C not in _prog_cache:
        _prog_cache[C] = build_program(C)
    return _prog_cache[C]


def _route(h32, gate_w):
    """Host router: returns per-expert (token_idx, combine_weight)."""
    logits = h32 @ np.asarray(gate_w, np.float32).T            # [T, E]
    rw = 1.0 / (1.0 + np.exp(-logits))
    topk_idx = np.argsort(-rw, axis=-1, kind="stable")[:, :TOPK]
    topk_w = np.take_along_axis(rw, topk_idx, -1)
    topk_w = topk_w / (topk_w.sum(-1, keepdims=True) + 1e-6) * SCALE
    sel, wsel = [], []
    for e in range(E):
        mask = topk_idx == e                                    # [T, K]
        tok = np.nonzero(mask.any(-1))[0]
        sel.append(tok)
        wsel.append((topk_w * mask).sum(-1)[tok].astype(np.float32))
    return sel, wsel


def kernel(hidden_states, gate_w, Wg, Wu, Wd, sWg, sWu, sWd):
    h32 = np.asarray(hidden_states, np.float32).reshape(T, H)
    sel, wsel = _route(h32, gate_w)

    C = max(P, int(math.ceil(max(len(s) for s in sel) / P)) * P)
    CT = C // P
    nc = _get_program(C)

    ht_bf = np.ascontiguousarray(h32.T).astype(BF16)            # [H, T]
    swg_bf = np.ascontiguousarray(np.asarray(sWg, np.float32).astype(BF16))
    swu_bf = np.ascontiguousarray(np.asarray(sWu, np.float32).astype(BF16))
    swd_bf = np.ascontiguousarray(np.asarray(sWd, np.float32).astype(BF16))
    wg_bf = np.asarray(Wg, np.float32).astype(BF16)
    wu_bf = np.asarray(Wu, np.float32).astype(BF16)
    wd_bf = np.asarray(Wd, np.float32).astype(BF16)

    in_maps = []
    for c in range(NCORES):
        xt = np.zeros((H, EL * C), BF16)
        wtk = np.zeros((P, EL * CT), np.float32)
        for le in range(EL):
            e = EL * c + le
            tok, w = sel[e], wsel[e]
            n = len(tok)
            xt[:, le * C:le * C + n] = ht_bf[:, tok]
            wcol = np.zeros(C, np.float32)
            wcol[:n] = w
            wtk[:, le * CT:(le + 1) * CT] = wcol.reshape(CT, P).T
        in_maps.append({
            "xt": xt,
            "wtk": wtk,
            "hts": np.ascontiguousarray(ht_bf[:, c * TSH:(c + 1) * TSH]),
            "wg": np.ascontiguousarray(wg_bf[EL * c:EL * (c + 1)]),
            "wu": np.ascontiguousarray(wu_bf[EL * c:EL * (c + 1)]),
            "wd": np.ascontiguousarray(wd_bf[EL * c:EL * (c + 1)]),
            "swg": swg_bf,
            "swu": swu_bf,
            "swd": swd_bf,
        })

    res = run_bass_kernel_spmd(nc, in_maps, list(range(NCORES)),
                               trace=TRACE, **TRACE_KWARGS)
    if TRACE:
        global LAST_RESULT
        LAST_RESULT = res

    out = np.empty((T, H), np.float32)
    for c in range(NCORES):
        out[c * TSH:(c + 1) * TSH] = res.results[c]["ysh"]
    for c in range(NCORES):
        yx = res.results[c]["yex"]
        for le in range(EL):
            e = EL * c + le
            tok = sel[e]
            out[tok] += yx[le, :len(tok)]

    return out.reshape(np.asarray(hidden_states).shape).astype(np.float32)


# revision 5
# speedup vs baseline: 1.2222x; 1.2222x over previous
"""DeepseekV3 MoE layer on 8 Trainium2 NeuronCores (Bass/Tile).

Strategy (expert-parallel, host-routed):
  - Router (h @ gate_w.T, sigmoid, top-8-of-16, weight norm) runs on host in
    fp32 — it is ~134 MFLOP, negligible, and data-dependent control flow is
    hostile to the static device ISA.
  - The 16 routed experts are sharded 2-per-core.  The host gathers each
    expert's tokens (capacity-padded to a multiple of 128), ships them
    transposed ([H, C] "feature-major") in bf16, and the device runs the
    fused SwiGLU chain  Y = (silu(X@Wg) * (X@Wu)) @ Wd  with fp32 PSUM
    accumulation, scaling rows by the combine weight on the way out.
  - The shared expert is token-parallel: each core computes the full shared
    MLP for its T/8 = 256 tokens (shared weights replicated, streamed).
  - Host scatters per-expert outputs back (indices are unique per expert)
    and adds the shared part.  All combination arithmetic is fp32.

Device layout notes:
  - All matmuls keep activations feature-major [feat, tokens] so that the
    gate/up projections need no transposes (weights are naturally [K, M])
    and the down projection consumes A^T directly as the stationary operand,
    returning token-major output tiles.
  - PSUM is managed as 8 single-bank [128, 512] fp32 tiles shared by every
    accumulation group.
"""

import math

import numpy as np
import ml_dtypes

import concourse.mybir as mybir
import concourse.tile as tile
from concourse import bacc
from concourse.bass_utils import run_bass_kernel_spmd

BF16 = ml_dtypes.bfloat16
F32 = mybir.dt.float32
BF = mybir.dt.bfloat16
ACT = mybir.ActivationFunctionType

H = 2048          # hidden size
IM = 1024         # routed expert intermediate
E = 16            # routed experts
TOPK = 8
T = 2048          # tokens (B=1, L=2048)
SCALE = 2.5
NCORES = 8
EL = E // NCORES  # experts per core
IS = 2048         # shared expert intermediate (IM * n_shared)
TSH = T // NCORES # shared-expert tokens per core
P = 128


def _chunks(total, size):
    return [(o, min(size, total - o)) for o in range(0, total, size)]


def build_program(C, h=H, im=IM, ishr=IS, tsh=TSH, el=EL, n_devices=NCORES):
    """Build + bass-compile the per-core SPMD program for capacity C."""
    assert C % P == 0 and h % 512 == 0 and im % P == 0 and ishr % P == 0
    KH = h // P            # contraction tiles over hidden dim
    MI = im // P           # routed intermediate partition tiles (<= 8)
    assert MI <= 8
    CT = C // P
    HN = h // 512
    ISM = ishr // P        # shared intermediate partition tiles
    nch = _chunks(C, 512)
    assert tsh <= 512 and tsh % P == 0

    nc = bacc.Bacc("TRN2", target_bir_lowering=False, debug=False,
                   num_devices=n_devices)

    xt = nc.dram_tensor("xt", [h, el * C], BF, kind="ExternalInput").ap()
    wtk = nc.dram_tensor("wtk", [P, el * CT], F32, kind="ExternalInput").ap()
    hts = nc.dram_tensor("hts", [h, tsh], BF, kind="ExternalInput").ap()
    wg = nc.dram_tensor("wg", [el, h, im], BF, kind="ExternalInput").ap()
    wu = nc.dram_tensor("wu", [el, h, im], BF, kind="ExternalInput").ap()
    wd = nc.dram_tensor("wd", [el, im, h], BF, kind="ExternalInput").ap()
    swg = nc.dram_tensor("swg", [h, ishr], BF, kind="ExternalInput").ap()
    swu = nc.dram_tensor("swu", [h, ishr], BF, kind="ExternalInput").ap()
    swd = nc.dram_tensor("swd", [ishr, h], BF, kind="ExternalInput").ap()
    yex = nc.dram_tensor("yex", [el, C, h], F32, kind="ExternalOutput").ap()
    ysh = nc.dram_tensor("ysh", [tsh, h], F32, kind="ExternalOutput").ap()

    QK = 4                 # hidden-dim k-slabs fetched per DMA
    assert KH % QK == 0
    with tile.TileContext(nc) as tc:
        with (
            tc.tile_pool(name="consts", bufs=1) as consts,
            tc.tile_pool(name="wpool", bufs=2) as wpool,
            tc.tile_pool(name="xpool", bufs=4) as xpool,
            tc.tile_pool(name="gpool", bufs=1) as gpool,
            tc.tile_pool(name="apool", bufs=2) as apool,
            tc.tile_pool(name="shw", bufs=3) as shw,
            tc.tile_pool(name="stage", bufs=3) as stage,
            tc.tile_pool(name="psum", bufs=8, space="PSUM") as psum,
        ):
            wtk_sb = consts.tile([P, el * CT], F32, name="wtk_sb")
            nc.sync.dma_start(wtk_sb[:], wtk[:, :])

            # ---------------- shared expert (token-parallel) --------------
            hts_sb = consts.tile([P, KH, tsh], BF, name="hts_sb")
            nc.sync.dma_start(hts_sb[:], hts.rearrange("(k p) t -> p k t", p=P))

            gss = consts.tile([P, ISM, tsh], BF, name="gss")
            ass = consts.tile([P, ISM, tsh], BF, name="ass")

            mgroups = _chunks(ISM, 8)
            for mat, is_gate in ((swg, True), (swu, False)):
                for (moff, msz) in mgroups:
                    pss = [psum.tile([P, 512], F32, name="ps", tag="ps")
                           for _ in range(msz)]
                    for k0 in range(KH // QK):
                        wsl = shw.tile([P, QK, 8 * P], BF, name="sl",
                                       tag="sl")[:, :, :msz * P]
                        nc.sync.dma_start(
                            wsl,
                            mat[k0 * QK * P:(k0 + 1) * QK * P,
                                moff * P:moff * P + msz * P]
                            .rearrange("(q p) m -> p q m", p=P))
                        for kq in range(QK):
                            k = k0 * QK + kq
                            for m in range(msz):
                                nc.tensor.matmul(
                                    pss[m][:, :tsh],
                                    wsl[:, kq, m * P:(m + 1) * P],
                                    hts_sb[:, k, :],
                                    start=(k == 0), stop=(k == KH - 1))
                    for m in range(msz):
                        mi = moff + m
                        if is_gate:
                            nc.scalar.activation(gss[:, mi, :],
                                                 pss[m][:, :tsh], ACT.Silu)
                        else:
                            nc.vector.tensor_mul(ass[:, mi, :], gss[:, mi, :],
                                                 pss[m][:, :tsh])

            # down-projection of the shared expert: token-major out
            psy = [[psum.tile([P, 512], F32, name="ps", tag="ps")
                    for _ in range(HN)] for _ in range(tsh // P)]
            QD = min(ISM, max(1, (QK * 8 * P) // h))  # k-slabs per DMA at h
            assert ISM % QD == 0
            for kk0 in range(ISM // QD):
                dsl = shw.tile([P, QD, h], BF, name="sl", tag="sl")
                nc.sync.dma_start(
                    dsl, swd[kk0 * QD * P:(kk0 + 1) * QD * P, :]
                    .rearrange("(q p) m -> p q m", p=P))
                for kq in range(QD):
                    kk = kk0 * QD + kq
                    for cm in range(tsh // P):
                        for hn in range(HN):
                            nc.tensor.matmul(
                                psy[cm][hn],
                                ass[:, kk, cm * P:(cm + 1) * P],
                                dsl[:, kq, hn * 512:(hn + 1) * 512],
                                start=(kk == 0), stop=(kk == ISM - 1))
            for cm in range(tsh // P):
                st = stage.tile([P, h], F32, name="st", tag="st")
                for hn in range(HN):
                    nc.scalar.activation(st[:, hn * 512:(hn + 1) * 512],
                                         psy[cm][hn][:], ACT.Copy)
                nc.sync.dma_start(ysh[cm * P:(cm + 1) * P, :], st[:])

            # ---------------- routed experts (expert-parallel) ------------
            for e in range(el):
                # gate projection
                wgt = wpool.tile([P, KH, im], BF, name="wmat", tag="wmat")
                nc.sync.dma_start(wgt[:],
                                  wg[e].rearrange("(k p) i -> p k i", p=P))
                gst = gpool.tile([P, MI, C], BF, name="gst")
                for (noff, nsz) in nch:
                    pse = [psum.tile([P, 512], F32, name="ps", tag="ps")
                           for _ in range(MI)]
                    for k0 in range(KH // QK):
                        xsl = xpool.tile([P, QK, 512], BF, name="xsl",
                                         tag="xsl")[:, :, :nsz]
                        nc.sync.dma_start(
                            xsl, xt[k0 * QK * P:(k0 + 1) * QK * P,
                                    e * C + noff:e * C + noff + nsz]
                            .rearrange("(q p) n -> p q n", p=P))
                        for kq in range(QK):
                            k = k0 * QK + kq
                            for m in range(MI):
                                nc.tensor.matmul(
                                    pse[m][:, :nsz],
                                    wgt[:, k, m * P:(m + 1) * P],
                                    xsl[:, kq, :],
                                    start=(k == 0), stop=(k == KH - 1))
                    for m in range(MI):
                        nc.scalar.activation(gst[:, m, noff:noff + nsz],
                                             pse[m][:, :nsz], ACT.Silu)

                # up projection (evicts through silu(G) * U)
                wut = wpool.tile([P, KH, im], BF, name="wmat", tag="wmat")
                nc.sync.dma_start(wut[:],
                                  wu[e].rearrange("(k p) i -> p k i", p=P))
                ast = apool.tile([P, MI, C], BF, name="ast")
                for (noff, nsz) in nch:
                    pse = [psum.tile([P, 512], F32, name="ps", tag="ps")
                           for _ in range(MI)]
                    for k0 in range(KH // QK):
                        xsl = xpool.tile([P, QK, 512], BF, name="xsl",
                                         tag="xsl")[:, :, :nsz]
                        nc.sync.dma_start(
                            xsl, xt[k0 * QK * P:(k0 + 1) * QK * P,
                                    e * C + noff:e * C + noff + nsz]
                            .rearrange("(q p) n -> p q n", p=P))
                        for kq in range(QK):
                            k = k0 * QK + kq
                            for m in range(MI):
                                nc.tensor.matmul(
                                    pse[m][:, :nsz],
                                    wut[:, k, m * P:(m + 1) * P],
                                    xsl[:, kq, :],
                                    start=(k == 0), stop=(k == KH - 1))
                    for m in range(MI):
                        nc.vector.tensor_mul(ast[:, m, noff:noff + nsz],
                                             gst[:, m, noff:noff + nsz],
                                             pse[m][:, :nsz])

                # down projection, token-major out, combine-weight scaling
                wdt = wpool.tile([P, MI, h], BF, name="wmat", tag="wmat")
                nc.sync.dma_start(wdt[:],
                                  wd[e].rearrange("(k p) i -> p k i", p=P))
                for cm in range(CT):
                    st = stage.tile([P, h], F32, name="st", tag="st")
                    for hn in range(HN):
                        pst = psum.tile([P, 512], F32, name="ps", tag="ps")
                        for kk in range(MI):
                            nc.tensor.matmul(
                                pst, ast[:, kk, cm * P:(cm + 1) * P],
                                wdt[:, kk, hn * 512:(hn + 1) * 512],
                                start=(kk == 0), stop=(kk == MI - 1))
                        nc.scalar.activation(
                            st[:, hn * 512:(hn + 1) * 512], pst[:], ACT.Copy,
                            scale=wtk_sb[:, e * CT + cm:e * CT + cm + 1])
                    nc.sync.dma_start(yex[e, cm * P:(cm + 1) * P, :], st[:])

    nc.compile()
    return nc


_prog_cache = {}

# Debug/timing hooks for the local test harness (harmless defaults for
# grading: no tracing, results kept only when asked for).
TRACE = False
TRACE_KWARGS = {}
LAST_RESULT = None


def _get_program(C):
    if C not in _prog_cache:
        _prog_cache[C] = build_program(C)
    return _prog_cache[C]


def _route(h32, gate_w):
    """Host router: returns per-expert (token_idx, combine_weight)."""
    logits = h32 @ np.asarray(gate_w, np.float32).T            # [T, E]
    rw = 1.0 / (1.0 + np.exp(-logits))
    topk_idx = np.argsort(-rw, axis=-1, kind="stable")[:, :TOPK]
    topk_w = np.take_along_axis(rw, topk_idx, -1)
    topk_w = topk_w / (topk_w.sum(-1, keepdims=True) + 1e-6) * SCALE
    sel, wsel = [], []
    for e in range(E):
        mask = topk_idx == e                                    # [T, K]
        tok = np.nonzero(mask.any(-1))[0]
        sel.append(tok)
        wsel.append((topk_w * mask).sum(-1)[tok].astype(np.float32))
    return sel, wsel


def kernel(hidden_states, gate_w, Wg, Wu, Wd, sWg, sWu, sWd):
    h32 = np.asarray(hidden_states, np.float32).reshape(T, H)
    sel, wsel = _route(h32, gate_w)

    C = max(P, int(math.ceil(max(len(s) for s in sel) / P)) * P)
    CT = C // P
    nc = _get_program(C)

    ht_bf = np.ascontiguousarray(h32.T).astype(BF16)            # [H, T]
    swg_bf = np.ascontiguousarray(np.asarray(sWg, np.float32).astype(BF16))
    swu_bf = np.ascontiguousarray(np.asarray(sWu, np.float32).astype(BF16))
    swd_bf = np.ascontiguousarray(np.asarray(sWd, np.float32).astype(BF16))
    wg_bf = np.asarray(Wg, np.float32).astype(BF16)
    wu_bf = np.asarray(Wu, np.float32).astype(BF16)
    wd_bf = np.asarray(Wd, np.float32).astype(BF16)

    in_maps = []
    for c in range(NCORES):
        xt = np.zeros((H, EL * C), BF16)
        wtk = np.zeros((P, EL * CT), np.float32)
        for le in range(EL):
            e = EL * c + le
            tok, w = sel[e], wsel[e]
            n = len(tok)
            xt[:, le * C:le * C + n] = ht_bf[:, tok]
            wcol = np.zeros(C, np.float32)
            wcol[:n] = w
            wtk[:, le * CT:(le + 1) * CT] = wcol.reshape(CT, P).T
        in_maps.append({
            "xt": xt,
            "wtk": wtk,
            "hts": np.ascontiguousarray(ht_bf[:, c * TSH:(c + 1) * TSH]),
            "wg": np.ascontiguousarray(wg_bf[EL * c:EL * (c + 1)]),
            "wu": np.ascontiguousarray(wu_bf[EL * c:EL * (c + 1)]),
            "wd": np.ascontiguousarray(wd_bf[EL * c:EL * (c + 1)]),
            "swg": swg_bf,
            "swu": swu_bf,
            "swd": swd_bf,
        })

    res = run_bass_kernel_spmd(nc, in_maps, list(range(NCORES)),
                               trace=TRACE, **TRACE_KWARGS)
    if TRACE:
        global LAST_RESULT
        LAST_RESULT = res

    out = np.empty((T, H), np.float32)
    for c in range(NCORES):
        out[c * TSH:(c + 1) * TSH] = res.results[c]["ysh"]
    for c in range(NCORES):
        yx = res.results[c]["yex"]
        for le in range(EL):
            e = EL * c + le
            tok = sel[e]
            out[tok] += yx[le, :len(tok)]

    return out.reshape(np.asarray(hidden_states).shape).astype(np.float32)


# revision 9
# speedup vs baseline: 1.2469x; 1.0202x over previous
"""DeepseekV3 MoE layer on 8 Trainium2 NeuronCores (Bass/Tile).

Strategy (expert-parallel, host-routed):
  - Router (h @ gate_w.T, sigmoid, top-8-of-16, weight norm) runs on host in
    fp32 — it is ~134 MFLOP, negligible, and data-dependent control flow is
    hostile to the static device ISA.
  - The 16 routed experts are sharded 2-per-core.  The host gathers each
    expert's tokens (capacity-padded to a multiple of 128), ships them
    transposed ([H, C] "feature-major") in bf16, and the device runs the
    fused SwiGLU chain  Y = (silu(X@Wg) * (X@Wu)) @ Wd  with fp32 PSUM
    accumulation, scaling rows by the combine weight on the way out.
  - The shared expert is token-parallel: each core computes the full shared
    MLP for its T/8 = 256 tokens (shared weights replicated, streamed).
  - Host scatters per-expert outputs back (indices are unique per expert)
    and adds the shared part.  All combination arithmetic is fp32.

Device layout notes:
  - All matmuls keep activations feature-major [feat, tokens] so that the
    gate/up projections need no transposes (weights are naturally [K, M])
    and the down projection consumes A^T directly as the stationary operand,
    returning token-major output tiles.
  - PSUM is managed as 8 single-bank [128, 512] fp32 tiles shared by every
    accumulation group.
"""

import math

import numpy as np
import ml_dtypes

import concourse.mybir as mybir
import concourse.tile as tile
from concourse import bacc
from concourse.bass_utils import run_bass_kernel_spmd

BF16 = ml_dtypes.bfloat16
F32 = mybir.dt.float32
BF = mybir.dt.bfloat16
ACT = mybir.ActivationFunctionType

H = 2048          # hidden size
IM = 1024         # routed expert intermediate
E = 16            # routed experts
TOPK = 8
T = 2048          # tokens (B=1, L=2048)
SCALE = 2.5
NCORES = 8
EL = E // NCORES  # experts per core
IS = 2048         # shared expert intermediate (IM * n_shared)
TSH = T // NCORES # shared-expert tokens per core
P = 128


def _chunks(total, size):
    return [(o, min(size, total - o)) for o in range(0, total, size)]


def build_program(C, h=H, im=IM, ishr=IS, tsh=TSH, el=EL, n_devices=NCORES):
    """Build + bass-compile the per-core SPMD program for capacity C."""
    assert C % P == 0 and h % 512 == 0 and im % P == 0 and ishr % P == 0
    KH = h // P            # contraction tiles over hidden dim
    MI = im // P           # routed intermediate partition tiles (<= 8)
    assert MI <= 8
    CT = C // P
    HN = h // 512
    ISM = ishr // P        # shared intermediate partition tiles
    nch = _chunks(C, 512)
    assert tsh <= 512 and tsh % P == 0

    nc = bacc.Bacc("TRN2", target_bir_lowering=False, debug=False,
                   num_devices=n_devices)

    xt = nc.dram_tensor("xt", [h, el * C], BF, kind="ExternalInput").ap()
    wtk = nc.dram_tensor("wtk", [P, el * CT], F32, kind="ExternalInput").ap()
    hts = nc.dram_tensor("hts", [h, tsh], BF, kind="ExternalInput").ap()
    wg = nc.dram_tensor("wg", [el, h, im], BF, kind="ExternalInput").ap()
    wu = nc.dram_tensor("wu", [el, h, im], BF, kind="ExternalInput").ap()
    wd = nc.dram_tensor("wd", [el, im, h], BF, kind="ExternalInput").ap()
    swg = nc.dram_tensor("swg", [h, ishr], BF, kind="ExternalInput").ap()
    swu = nc.dram_tensor("swu", [h, ishr], BF, kind="ExternalInput").ap()
    swd = nc.dram_tensor("swd", [ishr, h], BF, kind="ExternalInput").ap()
    yex = nc.dram_tensor("yex", [el, C, h], F32, kind="ExternalOutput").ap()
    ysh = nc.dram_tensor("ysh", [tsh, h], F32, kind="ExternalOutput").ap()

    QK = 4                 # hidden-dim k-slabs fetched per DMA
    assert KH % QK == 0
    with tile.TileContext(nc) as tc:
        with (
            tc.tile_pool(name="consts", bufs=1) as consts,
            tc.tile_pool(name="wpool", bufs=2) as wpool,
            tc.tile_pool(name="xpool", bufs=4) as xpool,
            tc.tile_pool(name="gpool", bufs=1) as gpool,
            tc.tile_pool(name="apool", bufs=1) as apool,
            tc.tile_pool(name="shw", bufs=6) as shw,
            tc.tile_pool(name="stage", bufs=4) as stage,
            tc.tile_pool(name="psum", bufs=8, space="PSUM") as psum,
        ):
            # PE warm-up: the HAM clock gate releases only after ~3.4us of
            # sustained matmul activity; burn idle DMA-wait time on a dummy
            # accumulation so the real work starts at 2.4 GHz.
            warm = consts.tile([P, 256], BF, name="warm")
            nc.vector.memset(warm[:], 0.0)
            ps_w = psum.tile([P, 512], F32, name="ps", tag="ps")
            for i in range(20):
                nc.tensor.matmul(ps_w[:, :256], warm[:, :P], warm[:],
                                 start=(i == 0), stop=(i == 19))
            nc.vector.tensor_copy(warm[:, :P], ps_w[:, :P])

            # ---------------- shared expert (token-parallel) --------------
            hts_sb = consts.tile([P, KH, tsh], BF, name="hts_sb")
            for q in range(KH // QK):
                nc.sync.dma_start(
                    hts_sb[:, q * QK:(q + 1) * QK, :],
                    hts[q * QK * P:(q + 1) * QK * P, :]
                    .rearrange("(k p) t -> p k t", p=P))

            wtk_sb = consts.tile([P, el * CT], F32, name="wtk_sb")
            nc.sync.dma_start(wtk_sb[:], wtk[:, :])

            gss = consts.tile([P, ISM, tsh], BF, name="gss")
            ass = consts.tile([P, ISM, tsh], BF, name="ass")

            mgroups = _chunks(ISM, 8)
            for mat, is_gate in ((swg, True), (swu, False)):
                for (moff, msz) in mgroups:
                    pss = [psum.tile([P, 512], F32, name="ps", tag="ps")
                           for _ in range(msz)]
                    for k0 in range(KH // QK):
                        wsl = shw.tile([P, QK, 8 * P], BF, name="sl",
                                       tag="sl")[:, :, :msz * P]
                        nc.sync.dma_start(
                            wsl,
                            mat[k0 * QK * P:(k0 + 1) * QK * P,
                                moff * P:moff * P + msz * P]
                            .rearrange("(q p) m -> p q m", p=P))
                        for kq in range(QK):
                            k = k0 * QK + kq
                            for m in range(msz):
                                nc.tensor.matmul(
                                    pss[m][:, :tsh],
                                    wsl[:, kq, m * P:(m + 1) * P],
                                    hts_sb[:, k, :],
                                    start=(k == 0), stop=(k == KH - 1))
                    for m in range(msz):
                        mi = moff + m
                        if is_gate:
                            nc.scalar.activation(gss[:, mi, :],
                                                 pss[m][:, :tsh], ACT.Silu)
                        else:
                            nc.vector.tensor_mul(ass[:, mi, :], gss[:, mi, :],
                                                 pss[m][:, :tsh])

            # down-projection of the shared expert: token-major out
            psy = [[psum.tile([P, 512], F32, name="ps", tag="ps")
                    for _ in range(HN)] for _ in range(tsh // P)]
            QD = min(ISM, max(1, (QK * 8 * P) // h))  # k-slabs per DMA at h
            assert ISM % QD == 0
            for kk0 in range(ISM // QD):
                dsl = shw.tile([P, QD, h], BF, name="sl", tag="sl")
                nc.sync.dma_start(
                    dsl, swd[kk0 * QD * P:(kk0 + 1) * QD * P, :]
                    .rearrange("(q p) m -> p q m", p=P))
                for kq in range(QD):
                    kk = kk0 * QD + kq
                    for cm in range(tsh // P):
                        for hn in range(HN):
                            nc.tensor.matmul(
                                psy[cm][hn],
                                ass[:, kk, cm * P:(cm + 1) * P],
                                dsl[:, kq, hn * 512:(hn + 1) * 512],
                                start=(kk == 0), stop=(kk == ISM - 1))
            for cm in range(tsh // P):
                for hn in range(HN):
                    st = stage.tile([P, 512], F32, name="st", tag="st")
                    nc.scalar.activation(st[:], psy[cm][hn][:], ACT.Copy)
                    nc.sync.dma_start(
                        ysh[cm * P:(cm + 1) * P, hn * 512:(hn + 1) * 512],
                        st[:])

            # ---------------- routed experts (expert-parallel) ------------
            for e in range(el):
                # gate projection
                wgt = wpool.tile([P, KH, im], BF, name="wmat", tag="wmat")
                nc.sync.dma_start(wgt[:],
                                  wg[e].rearrange("(k p) i -> p k i", p=P))
                gst = gpool.tile([P, MI, C], BF, name="gst")
                for (noff, nsz) in nch:
                    pse = [psum.tile([P, 512], F32, name="ps", tag="ps")
                           for _ in range(MI)]
                    for k0 in range(KH // QK):
                        xsl = xpool.tile([P, QK, 512], BF, name="xsl",
                                         tag="xsl")[:, :, :nsz]
                        nc.sync.dma_start(
                            xsl, xt[k0 * QK * P:(k0 + 1) * QK * P,
                                    e * C + noff:e * C + noff + nsz]
                            .rearrange("(q p) n -> p q n", p=P))
                        for kq in range(QK):
                            k = k0 * QK + kq
                            for m in range(MI):
                                nc.tensor.matmul(
                                    pse[m][:, :nsz],
                                    wgt[:, k, m * P:(m + 1) * P],
                                    xsl[:, kq, :],
                                    start=(k == 0), stop=(k == KH - 1))
                    for m in range(MI):
                        nc.scalar.activation(gst[:, m, noff:noff + nsz],
                                             pse[m][:, :nsz], ACT.Silu)

                # up projection (evicts through silu(G) * U)
                wut = wpool.tile([P, KH, im], BF, name="wmat", tag="wmat")
                nc.sync.dma_start(wut[:],
                                  wu[e].rearrange("(k p) i -> p k i", p=P))
                ast = apool.tile([P, MI, C], BF, name="ast")
                for (noff, nsz) in nch:
                    pse = [psum.tile([P, 512], F32, name="ps", tag="ps")
                           for _ in range(MI)]
                    for k0 in range(KH // QK):
                        xsl = xpool.tile([P, QK, 512], BF, name="xsl",
                                         tag="xsl")[:, :, :nsz]
                        nc.sync.dma_start(
                            xsl, xt[k0 * QK * P:(k0 + 1) * QK * P,
                                    e * C + noff:e * C + noff + nsz]
                            .rearrange("(q p) n -> p q n", p=P))
                        for kq in range(QK):
                            k = k0 * QK + kq
                            for m in range(MI):
                                nc.tensor.matmul(
                                    pse[m][:, :nsz],
                                    wut[:, k, m * P:(m + 1) * P],
                                    xsl[:, kq, :],
                                    start=(k == 0), stop=(k == KH - 1))
                    for m in range(MI):
                        nc.vector.tensor_mul(ast[:, m, noff:noff + nsz],
                                             gst[:, m, noff:noff + nsz],
                                             pse[m][:, :nsz])

                # down projection, token-major out, combine-weight scaling
                wdt = wpool.tile([P, MI, h], BF, name="wmat", tag="wmat")
                nc.sync.dma_start(wdt[:],
                                  wd[e].rearrange("(k p) i -> p k i", p=P))
                for cm in range(CT):
                    for hn in range(HN):
                        pst = psum.tile([P, 512], F32, name="ps", tag="ps")
                        for kk in range(MI):
                            nc.tensor.matmul(
                                pst, ast[:, kk, cm * P:(cm + 1) * P],
                                wdt[:, kk, hn * 512:(hn + 1) * 512],
                                start=(kk == 0), stop=(kk == MI - 1))
                        st = stage.tile([P, 512], F32, name="st", tag="st")
                        nc.scalar.activation(
                            st[:], pst[:], ACT.Copy,
                            scale=wtk_sb[:, e * CT + cm:e * CT + cm + 1])
                        nc.sync.dma_start(
                            yex[e, cm * P:(cm + 1) * P,
                                hn * 512:(hn + 1) * 512], st[:])

    nc.compile()
    return nc


_prog_cache = {}

# Debug/timing hooks for the local test harness (harmless defaults for
# grading: no tracing, results kept only when asked for).
TRACE = False
TRACE_KWARGS = {}
LAST_RESULT = None


def _get_program(C):
    if C not in _prog_cache:
        _prog_cache[C] = build_program(C)
    return _prog_cache[C]


def _route(h32, gate_w):
    """Host router: returns per-expert (token_idx, combine_weight)."""
    logits = h32 @ np.asarray(gate_w, np.float32).T            # [T, E]
    rw = 1.0 / (1.0 + np.exp(-logits))
    topk_idx = np.argsort(-rw, axis=-1, kind="stable")[:, :TOPK]
    topk_w = np.take_along_axis(rw, topk_idx, -1)
    topk_w = topk_w / (topk_w.sum(-1, keepdims=True) + 1e-6) * SCALE
    sel, wsel = [], []
    for e in range(E):
        mask = topk_idx == e                                    # [T, K]
        tok = np.nonzero(mask.any(-1))[0]
        sel.append(tok)
        wsel.append((topk_w * mask).sum(-1)[tok].astype(np.float32))
    return sel, wsel


def kernel(hidden_states, gate_w, Wg, Wu, Wd, sWg, sWu, sWd):
    h32 = np.asarray(hidden_states, np.float32).reshape(T, H)
    sel, wsel = _route(h32, gate_w)

    C = max(P, int(math.ceil(max(len(s) for s in sel) / P)) * P)
    CT = C // P
    nc = _get_program(C)

    ht_bf = np.ascontiguousarray(h32.T).astype(BF16)            # [H, T]
    swg_bf = np.ascontiguousarray(np.asarray(sWg, np.float32).astype(BF16))
    swu_bf = np.ascontiguousarray(np.asarray(sWu, np.float32).astype(BF16))
    swd_bf = np.ascontiguousarray(np.asarray(sWd, np.float32).astype(BF16))
    wg_bf = np.asarray(Wg, np.float32).astype(BF16)
    wu_bf = np.asarray(Wu, np.float32).astype(BF16)
    wd_bf = np.asarray(Wd, np.float32).astype(BF16)

    in_maps = []
    for c in range(NCORES):
        xt = np.zeros((H, EL * C), BF16)
        wtk = np.zeros((P, EL * CT), np.float32)
        for le in range(EL):
            e = EL * c + le
            tok, w = sel[e], wsel[e]
            n = len(tok)
            xt[:, le * C:le * C + n] = ht_bf[:, tok]
            wcol = np.zeros(C, np.float32)
            wcol[:n] = w
            wtk[:, le * CT:(le + 1) * CT] = wcol.reshape(CT, P).T
        in_maps.append({
            "xt": xt,
            "wtk": wtk,
            "hts": np.ascontiguousarray(ht_bf[:, c * TSH:(c + 1) * TSH]),
            "wg": np.ascontiguousarray(wg_bf[EL * c:EL * (c + 1)]),
            "wu": np.ascontiguousarray(wu_bf[EL * c:EL * (c + 1)]),
            "wd": np.ascontiguousarray(wd_bf[EL * c:EL * (c + 1)]),
            "swg": swg_bf,
            "swu": swu_bf,
            "swd": swd_bf,
        })

    res = run_bass_kernel_spmd(nc, in_maps, list(range(NCORES)),
                               trace=TRACE, **TRACE_KWARGS)
    if TRACE:
        global LAST_RESULT
        LAST_RESULT = res

    out = np.empty((T, H), np.float32)
    for c in range(NCORES):
        out[c * TSH:(c + 1) * TSH] = res.results[c]["ysh"]
    for c in range(NCORES):
        yx = res.results[c]["yex"]
        for le in range(EL):
            e = EL * c + le
            tok = sel[e]
            out[tok] += yx[le, :len(tok)]

    return out.reshape(np.asarray(hidden_states).shape).astype(np.float32)


# revision 10
# speedup vs baseline: 1.2662x; 1.0155x over previous
"""DeepseekV3 MoE layer on 8 Trainium2 NeuronCores (Bass/Tile).

Strategy (expert-parallel, host-routed):
  - Router (h @ gate_w.T, sigmoid, top-8-of-16, weight norm) runs on host in
    fp32 — it is ~134 MFLOP, negligible, and data-dependent control flow is
    hostile to the static device ISA.
  - The 16 routed experts are sharded 2-per-core.  The host gathers each
    expert's tokens (capacity-padded to a multiple of 128), ships them
    transposed ([H, C] "feature-major") in bf16, and the device runs the
    fused SwiGLU chain  Y = (silu(X@Wg) * (X@Wu)) @ Wd  with fp32 PSUM
    accumulation, scaling rows by the combine weight on the way out.
    Experts are assigned to the two per-core slots by descending token
    count, so slot 0 compiles with capacity C0 >= C1 of slot 1 and the
    padding waste stays small.
  - The shared expert is token-parallel: each core computes the full shared
    MLP for its T/8 = 256 tokens (shared weights replicated, streamed).
  - Host scatters per-expert outputs back (indices are unique per expert)
    and adds the shared part.  All combination arithmetic is fp32.

Device notes:
  - Activations stay feature-major [feat, tokens] so gate/up projections
    need no transposes and the down projection consumes A^T directly as the
    stationary operand, returning token-major output tiles.
  - PSUM is managed as 8 single-bank [128, 512] fp32 tiles shared by every
    accumulation group.
  - DMA issue is split across the two HWDGE engines (sync + scalar) —
    each engine owns one hardware queue, so this doubles queue parallelism
    and keeps the shared-expert stream off the weight-fetch path.
"""

import math

import numpy as np
import ml_dtypes

import concourse.mybir as mybir
import concourse.tile as tile
from concourse import bacc
from concourse.bass_utils import run_bass_kernel_spmd

BF16 = ml_dtypes.bfloat16
F32 = mybir.dt.float32
BF = mybir.dt.bfloat16
ACT = mybir.ActivationFunctionType

H = 2048          # hidden size
IM = 1024         # routed expert intermediate
E = 16            # routed experts
TOPK = 8
T = 2048          # tokens (B=1, L=2048)
SCALE = 2.5
NCORES = 8
EL = E // NCORES  # experts per core
IS = 2048         # shared expert intermediate (IM * n_shared)
TSH = T // NCORES # shared-expert tokens per core
P = 128


def _chunks(total, size):
    return [(o, min(size, total - o)) for o in range(0, total, size)]


def build_program(caps, h=H, im=IM, ishr=IS, tsh=TSH, n_devices=NCORES):
    """Build + bass-compile the per-core SPMD program.

    caps: per-expert-slot token capacities (multiples of 128), len == EL.
    """
    el = len(caps)
    assert all(c % P == 0 for c in caps)
    assert h % 512 == 0 and im % P == 0 and ishr % P == 0
    KH = h // P            # contraction tiles over hidden dim
    MI = im // P           # routed intermediate partition tiles (<= 8)
    assert MI <= 8
    HN = h // 512
    ISM = ishr // P        # shared intermediate partition tiles
    assert tsh <= 512 and tsh % P == 0
    CSUM = sum(caps)
    coff = [sum(caps[:i]) for i in range(el)]          # xt column offsets
    CT = [c // P for c in caps]
    ctoff = [sum(CT[:i]) for i in range(el)]           # wtk column offsets

    nc = bacc.Bacc("TRN2", target_bir_lowering=False, debug=False,
                   num_devices=n_devices)

    xt = nc.dram_tensor("xt", [h, CSUM], BF, kind="ExternalInput").ap()
    wtk = nc.dram_tensor("wtk", [P, sum(CT)], F32, kind="ExternalInput").ap()
    hts = nc.dram_tensor("hts", [h, tsh], BF, kind="ExternalInput").ap()
    wg = nc.dram_tensor("wg", [el, h, im], BF, kind="ExternalInput").ap()
    wu = nc.dram_tensor("wu", [el, h, im], BF, kind="ExternalInput").ap()
    wd = nc.dram_tensor("wd", [el, im, h], BF, kind="ExternalInput").ap()
    swg = nc.dram_tensor("swg", [h, ishr], BF, kind="ExternalInput").ap()
    swu = nc.dram_tensor("swu", [h, ishr], BF, kind="ExternalInput").ap()
    swd = nc.dram_tensor("swd", [ishr, h], BF, kind="ExternalInput").ap()
    yex = [nc.dram_tensor(f"yex{e}", [caps[e], h], F32,
                          kind="ExternalOutput").ap() for e in range(el)]
    ysh = nc.dram_tensor("ysh", [tsh, h], F32, kind="ExternalOutput").ap()

    QK = 4                 # hidden-dim k-slabs fetched per DMA
    assert KH % QK == 0
    with tile.TileContext(nc) as tc:
        with (
            tc.tile_pool(name="consts", bufs=1) as consts,
            tc.tile_pool(name="wpool", bufs=2) as wpool,
            tc.tile_pool(name="xpool", bufs=4) as xpool,
            tc.tile_pool(name="gpool", bufs=1) as gpool,
            tc.tile_pool(name="apool", bufs=1) as apool,
            tc.tile_pool(name="shw", bufs=6) as shw,
            tc.tile_pool(name="stage", bufs=4) as stage,
            tc.tile_pool(name="psum", bufs=8, space="PSUM") as psum,
        ):
            # PE warm-up: the HAM clock gate releases only after ~3.4us of
            # sustained matmul activity; burn the initial DMA-wait time on a
            # dummy accumulation so real work starts at 2.4 GHz.
            warm = consts.tile([P, 256], BF, name="warm")
            nc.vector.memset(warm[:], 0.0)
            ps_w = psum.tile([P, 512], F32, name="ps", tag="ps")
            for i in range(20):
                nc.tensor.matmul(ps_w[:, :256], warm[:, :P], warm[:],
                                 start=(i == 0), stop=(i == 19))
            nc.vector.tensor_copy(warm[:, :P], ps_w[:, :P])

            # ---------------- shared expert (token-parallel) --------------
            # its input stream is issued by the scalar engine (own HW queue)
            hts_sb = consts.tile([P, KH, tsh], BF, name="hts_sb")
            for q in range(KH // QK):
                nc.scalar.dma_start(
                    hts_sb[:, q * QK:(q + 1) * QK, :],
                    hts[q * QK * P:(q + 1) * QK * P, :]
                    .rearrange("(k p) t -> p k t", p=P))

            wtk_sb = consts.tile([P, sum(CT)], F32, name="wtk_sb")
            nc.scalar.dma_start(wtk_sb[:], wtk[:, :])

            gss = consts.tile([P, ISM, tsh], BF, name="gss")
            ass = consts.tile([P, ISM, tsh], BF, name="ass")

            mgroups = _chunks(ISM, 8)
            for mat, is_gate in ((swg, True), (swu, False)):
                for (moff, msz) in mgroups:
                    pss = [psum.tile([P, 512], F32, name="ps", tag="ps")
                           for _ in range(msz)]
                    for k0 in range(KH // QK):
                        wsl = shw.tile([P, QK, 8 * P], BF, name="sl",
                                       tag="sl")[:, :, :msz * P]
                        nc.scalar.dma_start(
                            wsl,
                            mat[k0 * QK * P:(k0 + 1) * QK * P,
                                moff * P:moff * P + msz * P]
                            .rearrange("(q p) m -> p q m", p=P))
                        for kq in range(QK):
                            k = k0 * QK + kq
                            for m in range(msz):
                                nc.tensor.matmul(
                                    pss[m][:, :tsh],
                                    wsl[:, kq, m * P:(m + 1) * P],
                                    hts_sb[:, k, :],
                                    start=(k == 0), stop=(k == KH - 1))
                    for m in range(msz):
                        mi = moff + m
                        if is_gate:
                            nc.scalar.activation(gss[:, mi, :],
                                                 pss[m][:, :tsh], ACT.Silu)
                        else:
                            nc.vector.tensor_mul(ass[:, mi, :], gss[:, mi, :],
                                                 pss[m][:, :tsh])

            # down-projection of the shared expert: token-major out
            psy = [[psum.tile([P, 512], F32, name="ps", tag="ps")
                    for _ in range(HN)] for _ in range(tsh // P)]
            QD = min(ISM, max(1, (QK * 8 * P) // h))  # k-slabs per DMA at h
            assert ISM % QD == 0
            for kk0 in range(ISM // QD):
                dsl = shw.tile([P, QD, h], BF, name="sl", tag="sl")
                nc.scalar.dma_start(
                    dsl, swd[kk0 * QD * P:(kk0 + 1) * QD * P, :]
                    .rearrange("(q p) m -> p q m", p=P))
                for kq in range(QD):
                    kk = kk0 * QD + kq
                    for cm in range(tsh // P):
                        for hn in range(HN):
                            nc.tensor.matmul(
                                psy[cm][hn],
                                ass[:, kk, cm * P:(cm + 1) * P],
                                dsl[:, kq, hn * 512:(hn + 1) * 512],
                                start=(kk == 0), stop=(kk == ISM - 1))
            for cm in range(tsh // P):
                for hn in range(HN):
                    st = stage.tile([P, 512], F32, name="st", tag="st")
                    nc.scalar.activation(st[:], psy[cm][hn][:], ACT.Copy)
                    nc.sync.dma_start(
                        ysh[cm * P:(cm + 1) * P, hn * 512:(hn + 1) * 512],
                        st[:])

            # ---------------- routed experts (expert-parallel) ------------
            for e in range(el):
                C = caps[e]
                nch = _chunks(C, 512)
                # gate projection
                wgt = wpool.tile([P, KH, im], BF, name="wmat", tag="wmat")
                nc.sync.dma_start(wgt[:],
                                  wg[e].rearrange("(k p) i -> p k i", p=P))
                gst = gpool.tile([P, MI, caps[0]], BF, name="gst")
                for (noff, nsz) in nch:
                    pse = [psum.tile([P, 512], F32, name="ps", tag="ps")
                           for _ in range(MI)]
                    for k0 in range(KH // QK):
                        xsl = xpool.tile([P, QK, 512], BF, name="xsl",
                                         tag="xsl")[:, :, :nsz]
                        nc.sync.dma_start(
                            xsl, xt[k0 * QK * P:(k0 + 1) * QK * P,
                                    coff[e] + noff:coff[e] + noff + nsz]
                            .rearrange("(q p) n -> p q n", p=P))
                        for kq in range(QK):
                            k = k0 * QK + kq
                            for m in range(MI):
                                nc.tensor.matmul(
                                    pse[m][:, :nsz],
                                    wgt[:, k, m * P:(m + 1) * P],
                                    xsl[:, kq, :],
                                    start=(k == 0), stop=(k == KH - 1))
                    for m in range(MI):
                        nc.scalar.activation(gst[:, m, noff:noff + nsz],
                                             pse[m][:, :nsz], ACT.Silu)

                # up projection (evicts through silu(G) * U)
                wut = wpool.tile([P, KH, im], BF, name="wmat", tag="wmat")
                nc.sync.dma_start(wut[:],
                                  wu[e].rearrange("(k p) i -> p k i", p=P))
                ast = apool.tile([P, MI, caps[0]], BF, name="ast")
                for (noff, nsz) in nch:
                    pse = [psum.tile([P, 512], F32, name="ps", tag="ps")
                           for _ in range(MI)]
                    for k0 in range(KH // QK):
                        xsl = xpool.tile([P, QK, 512], BF, name="xsl",
                                         tag="xsl")[:, :, :nsz]
                        nc.sync.dma_start(
                            xsl, xt[k0 * QK * P:(k0 + 1) * QK * P,
                                    coff[e] + noff:coff[e] + noff + nsz]
                            .rearrange("(q p) n -> p q n", p=P))
                        for kq in range(QK):
                            k = k0 * QK + kq
                            for m in range(MI):
                                nc.tensor.matmul(
                                    pse[m][:, :nsz],
                                    wut[:, k, m * P:(m + 1) * P],
                                    xsl[:, kq, :],
                                    start=(k == 0), stop=(k == KH - 1))
                    for m in range(MI):
                        nc.vector.tensor_mul(ast[:, m, noff:noff + nsz],
                                             gst[:, m, noff:noff + nsz],
                                             pse[m][:, :nsz])

                # down projection, token-major out, combine-weight scaling
                wdt = wpool.tile([P, MI, h], BF, name="wmat", tag="wmat")
                nc.sync.dma_start(wdt[:],
                                  wd[e].rearrange("(k p) i -> p k i", p=P))
                for cm in range(C // P):
                    for hn in range(HN):
                        pst = psum.tile([P, 512], F32, name="ps", tag="ps")
                        for kk in range(MI):
                            nc.tensor.matmul(
                                pst, ast[:, kk, cm * P:(cm + 1) * P],
                                wdt[:, kk, hn * 512:(hn + 1) * 512],
                                start=(kk == 0), stop=(kk == MI - 1))
                        st = stage.tile([P, 512], F32, name="st", tag="st")
                        nc.scalar.activation(
                            st[:], pst[:], ACT.Copy,
                            scale=wtk_sb[:, ctoff[e] + cm:ctoff[e] + cm + 1])
                        nc.sync.dma_start(
                            yex[e][cm * P:(cm + 1) * P,
                                   hn * 512:(hn + 1) * 512], st[:])

    nc.compile()
    return nc


_prog_cache = {}

# Debug/timing hooks for the local test harness (harmless defaults for
# grading: no tracing, results kept only when asked for).
TRACE = False
TRACE_KWARGS = {}
LAST_RESULT = None


def _get_program(caps):
    if caps not in _prog_cache:
        _prog_cache[caps] = build_program(caps)
    return _prog_cache[caps]


def _route(h32, gate_w):
    """Host router: returns per-expert (token_idx, combine_weight)."""
    logits = h32 @ np.asarray(gate_w, np.float32).T            # [T, E]
    rw = 1.0 / (1.0 + np.exp(-logits))
    topk_idx = np.argsort(-rw, axis=-1, kind="stable")[:, :TOPK]
    topk_w = np.take_along_axis(rw, topk_idx, -1)
    topk_w = topk_w / (topk_w.sum(-1, keepdims=True) + 1e-6) * SCALE
    sel, wsel = [], []
    for e in range(E):
        mask = topk_idx == e                                    # [T, K]
        tok = np.nonzero(mask.any(-1))[0]
        sel.append(tok)
        wsel.append((topk_w * mask).sum(-1)[tok].astype(np.float32))
    return sel, wsel


def kernel(hidden_states, gate_w, Wg, Wu, Wd, sWg, sWu, sWd):
    h32 = np.asarray(hidden_states, np.float32).reshape(T, H)
    sel, wsel = _route(h32, gate_w)

    # Assign experts to (core, slot): slot 0 gets the 8 busiest experts so
    # slot capacities (compile-time constants) hug the actual counts.
    order = sorted(range(E), key=lambda e: -len(sel[e]))
    slot_experts = [order[:NCORES], order[NCORES:]]             # [slot][core]
    caps = tuple(
        max(P, int(math.ceil(max(len(sel[e]) for e in slot_experts[s]) / P))
            * P)
        for s in range(EL))
    nc = _get_program(caps)
    coff = [sum(caps[:i]) for i in range(EL)]
    CT = [c // P for c in caps]
    ctoff = [sum(CT[:i]) for i in range(EL)]

    ht_bf = np.ascontiguousarray(h32.T).astype(BF16)            # [H, T]
    swg_bf = np.ascontiguousarray(np.asarray(sWg, np.float32).astype(BF16))
    swu_bf = np.ascontiguousarray(np.asarray(sWu, np.float32).astype(BF16))
    swd_bf = np.ascontiguousarray(np.asarray(sWd, np.float32).astype(BF16))
    wg_bf = np.asarray(Wg, np.float32).astype(BF16)
    wu_bf = np.asarray(Wu, np.float32).astype(BF16)
    wd_bf = np.asarray(Wd, np.float32).astype(BF16)

    in_maps = []
    for c in range(NCORES):
        experts = [slot_experts[s][c] for s in range(EL)]
        xt = np.zeros((H, sum(caps)), BF16)
        wtk = np.zeros((P, sum(CT)), np.float32)
        for s, e in enumerate(experts):
            tok, w = sel[e], wsel[e]
            n = len(tok)
            xt[:, coff[s]:coff[s] + n] = ht_bf[:, tok]
            wcol = np.zeros(caps[s], np.float32)
            wcol[:n] = w
            wtk[:, ctoff[s]:ctoff[s] + CT[s]] = wcol.reshape(CT[s], P).T
        in_maps.append({
            "xt": xt,
            "wtk": wtk,
            "hts": np.ascontiguousarray(ht_bf[:, c * TSH:(c + 1) * TSH]),
            "wg": np.ascontiguousarray(wg_bf[experts]),
            "wu": np.ascontiguousarray(wu_bf[experts]),
            "wd": np.ascontiguousarray(wd_bf[experts]),
            "swg": swg_bf,
            "swu": swu_bf,
            "swd": swd_bf,
        })

    res = run_bass_kernel_spmd(nc, in_maps, list(range(NCORES)),
                               trace=TRACE, **TRACE_KWARGS)
    if TRACE:
        global LAST_RESULT
        LAST_RESULT = res

    out = np.empty((T, H), np.float32)
    for c in range(NCORES):
        out[c * TSH:(c + 1) * TSH] = res.results[c]["ysh"]
    for c in range(NCORES):
        for s in range(EL):
            e = slot_experts[s][c]
            tok = sel[e]
            out[tok] += res.results[c][f"yex{s}"][:len(tok)]

    return out.reshape(np.asarray(hidden_states).shape).astype(np.float32)
